# revision 66
# baseline (speedup 1.0000x reference)
"""Trainium2 Bass kernel for knn_interpolate(K=3) + ResMLP over B=8 point clouds.

Sharding: data-parallel, one cloud per NeuronCore (8 cores).

Per-core pipeline (64 target tiles of 128 targets, software-pipelined in
groups of 8 tiles so each group's select/gather/refine/MLP phases overlap
the next group's scoring):
  A. scores[t,s] = pt.ps - |ps|^2/2 via bf16x2-split matmul into PSUM f32
     ([P,1024] halves, double-buffered; per-target bias unnecessary:
     per-partition shifts don't change per-target ranking).
  B. strided f32 tensor_reduce over each PSUM half -> 128 window maxima
     (half-contained windows: window j = sources {j%64 + 64m + 1024*[j>=64]},
     so the two half reduces fill disjoint halves of the window array and no
     combine op is needed); DVE Max8/MaxIndex8 picks the top-4 windows
     (offline-verified: exact top-3 always inside the top-4 windows).
  C. dma_gather of the 4 selected windows' coord blocks (host-permuted
     ps_win rows: 16 members x 16B = 256B contiguous per window); wrapped
     int16 index tables built via a q-major DRAM roundtrip (one write, one
     stride-0 broadcast read) + gpsimd interleave.
  D. exact fp32 d2 recompute in the reference op order ((dx^2+dy^2)+dz^2)
     over the 64 candidates; Max8/MaxIndex8 -> top-3 exact neighbors +
     inverse-d2 weights; slot -> source index via f32 mask arithmetic.
  E. dma_gather of the 3 selected source feature rows (bf16, 512B elems);
     weights applied by the PE itself using diag(wn) bf16 rhs blocks
     (built on ACT from a resident identity), accumulated over K into
     interp^T in PSUM -- no per-pair ACT prescale on the critical chain.
  F. channel-major ResMLP on tile pairs (N=256, float32r matmuls).
Host does layout-only prep (transposes / bf16 splits / |ps|^2 / window
permutation of ps) and the final unshard (channel-major -> row-major).
"""

import os
import sys

for _p in ("/opt/trn_rl_repo", "/root/.axon_site/_ro/trn_rl_repo"):
    if _p not in sys.path and os.path.isdir(_p):
        sys.path.insert(0, _p)

import numpy as np
import ml_dtypes

B = 8
NT = 8192
NS = 2048
C_TGT = 128
C_SRC = 256
C_HID = 256
C_OUT = 128
P = 128
K = 3

NW = 4                # windows gathered per target
WSZ = 16              # sources per window
NWIN = NS // WSZ      # 128 windows
NC = NW * WSZ         # 64 exact-refine candidates per target

TT = NT // P          # 64 target tiles per core
GROUP = 8             # tiles per pipeline group
FG = 8                # tiles per feature-gather buffer


def _bf16_split(x):
    hi = np.asarray(x, ml_dtypes.bfloat16)
    lo = np.asarray(x - hi.astype(np.float32), ml_dtypes.bfloat16)
    return hi, lo


def build_program(tt=TT):
    import concourse.bacc as bacc
    import concourse.mybir as mybir
    import concourse.tile as tile
    from concourse import bass

    f32 = mybir.dt.float32
    f32r = mybir.dt.float32r
    f16 = mybir.dt.float16
    bf16 = mybir.dt.bfloat16
    u16 = mybir.dt.uint16
    i16 = mybir.dt.int16
    Alu = mybir.AluOpType
    Act = mybir.ActivationFunctionType

    nc = bacc.Bacc("TRN2", debug=False, num_devices=8)
    nt = tt * P
    G = min(GROUP, tt)
    n_grp = tt // G
    fg = min(FG, G)

    # ---- DRAM tensors ----
    d_lhsT = nc.dram_tensor("lhsT_pt", [12, nt], bf16, kind="ExternalInput").ap()
    d_rhs = nc.dram_tensor("rhs_ps", [12, NS], bf16, kind="ExternalInput").ap()
    d_ptT = nc.dram_tensor("ptT", [P, tt * 3], f32, kind="ExternalInput").ap()
    d_xtT = nc.dram_tensor("xtT", [C_TGT, nt], f32r, kind="ExternalInput").ap()
    d_pswin = nc.dram_tensor("ps_win", [NWIN, WSZ * 4], f32, kind="ExternalInput").ap()
    d_xs = nc.dram_tensor("xs", [NS, C_SRC], bf16, kind="ExternalInput").ap()
    d_w1 = nc.dram_tensor("w1t", [P, 3 * 2 * P], f32r, kind="ExternalInput").ap()
    d_w2 = nc.dram_tensor("w2t", [P, 2 * P], f32r, kind="ExternalInput").ap()
    d_ws = nc.dram_tensor("wst", [P, 3 * P], f32r, kind="ExternalInput").ap()
    d_b1 = nc.dram_tensor("b1t", [P, 2], f32, kind="ExternalInput").ap()
    d_bo = nc.dram_tensor("bot", [P, 1], f32, kind="ExternalInput").ap()
    d_ident = nc.dram_tensor("ident", [P, P], f32, kind="ExternalInput").ap()
    d_id3b = nc.dram_tensor("id3b", [P, 3 * P], bf16, kind="ExternalInput").ap()
    d_nptq = nc.dram_tensor("nptq", [P, tt], f32, kind="ExternalInput").ap()
    d_nptc = nc.dram_tensor("nptc", [P, tt * 3], f32, kind="ExternalInput").ap()
    d_out = nc.dram_tensor("outT", [C_OUT, nt], f32, kind="ExternalOutput").ap()
    d_scr_c = nc.dram_tensor("scr_c", [n_grp, 16, 8 * NW * G], i16, kind="Internal").ap()
    d_scr_f = nc.dram_tensor("scr_f", [n_grp, 16, 8 * G * K], i16, kind="Internal").ap()

    with tile.TileContext(nc) as tc:
        with (
            tc.tile_pool(name="const", bufs=1) as cpool,
            tc.tile_pool(name="sel", bufs=1) as selpool,
            tc.tile_pool(name="psum_s", bufs=2, space="PSUM") as pspool,
            tc.tile_pool(name="tree", bufs=4) as tpool,
            tc.tile_pool(name="ssb", bufs=3) as spool,
            tc.tile_pool(name="gath", bufs=1) as gpool,
            tc.tile_pool(name="mlp", bufs=4) as mpool,
            tc.tile_pool(name="psum_m", bufs=1, space="PSUM") as psm,
        ):
            # ---- resident constants ----
            lhsT = cpool.tile([12, nt], bf16)
            nc.sync.dma_start(lhsT[:], d_lhsT)
            rhs = cpool.tile([12, NS], bf16)
            nc.sync.dma_start(rhs[:], d_rhs)
            ptT = cpool.tile([P, tt * 3], f32)
            nc.sync.dma_start(ptT[:], d_ptT)
            w1 = cpool.tile([P, 3 * 2 * P], f32r)
            nc.sync.dma_start(w1[:], d_w1)
            w2 = cpool.tile([P, 2 * P], f32r)
            nc.sync.dma_start(w2[:], d_w2)
            ws = cpool.tile([P, 3 * P], f32r)
            nc.sync.dma_start(ws[:], d_ws)
            b1 = cpool.tile([P, 2], f32)
            nc.sync.dma_start(b1[:], d_b1)
            bo = cpool.tile([P, 1], f32)
            nc.sync.dma_start(bo[:], d_bo)
            ident = cpool.tile([P, P], f32)
            nc.sync.dma_start(ident[:], d_ident)
            id3b = cpool.tile([P, 3 * P], bf16)
            nc.sync.dma_start(id3b[:], d_id3b)
            nptq = cpool.tile([P, tt], f32)
            nc.sync.dma_start(nptq[:], d_nptq)
            nptc = cpool.tile([P, tt * 3], f32)
            nc.sync.dma_start(nptc[:], d_nptc)

            # ---- per-group selection buffers, parity-paired ----
            widx8p = [selpool.tile([P, G, 8], u16, name=f"widx8{x}") for x in "ab"]
            snd2p = [selpool.tile([P, G, 8], f32, name=f"snd2{x}") for x in "ab"]
            slotsp = [selpool.tile([P, G, 8], u16, name=f"slots{x}") for x in "ab"]
            wnp = [selpool.tile([P, G, 3], f32, name=f"wn{x}") for x in "ab"]
            # group-scope scratch (reused each group)
            w3 = selpool.tile([P, G, K], f32)
            sumw = selpool.tile([P, G], f32)
            rsum = selpool.tile([P, G], f32)
            slotf = selpool.tile([P, G, K], f32)
            tsel = selpool.tile([P, G, K], f32)
            c1m = selpool.tile([P, G, K], f32)
            c2m = selpool.tile([P, G, K], f32)
            accw = selpool.tile([P, G, K], f32)
            widxf = selpool.tile([P, G, NW], f32)
            widxadj = selpool.tile([P, G, NW], f32)
            srcf = selpool.tile([P, G, K], f32)
            srci_a = selpool.tile([P, G * K], i16)
            srci_b = selpool.tile([P, G * K], i16)
            srci = [srci_a, srci_b]

            ptc3 = ptT.rearrange("p (t c) -> p t c", c=3)
            nptq3 = nptc.rearrange("p (t c) -> p t c", c=3)

            for g in range(n_grp):
                g0 = g * G
                gsl = slice(g0, g0 + G)
                # ============ Phase A/B: scores + window tree (per tile) ====
                for i in range(g0, g0 + G):
                    # half-tile scoring: [P,1024] PSUM double-buffered so the
                    # next half's matmuls overlap this half's window reduce
                    # (DVE may read only one non-scalar PSUM input)
                    ewin = tpool.tile([P, NWIN], f32, tag="ewin")
                    for h in range(2):
                        ps_s = pspool.tile([P, NS // 2], f32, tag="scores", bufs=2)
                        with tc.high_priority():
                            for n in range(2):
                                nc.tensor.matmul(
                                    ps_s[:, n * 512:(n + 1) * 512],
                                    lhsT=lhsT[:, i * P:(i + 1) * P],
                                    rhs=rhs[:, h * 1024 + n * 512:
                                            h * 1024 + (n + 1) * 512],
                                    start=True, stop=True,
                                )
                        # half-contained windows: window j = sources
                        # {j%64 + 64m + 1024*(j>=64)} -- each window lives in
                        # one PSUM half, so the reduces fill disjoint halves
                        # of ewin and no combine op is needed
                        nc.vector.tensor_reduce(
                            out=ewin[:, h * 64:(h + 1) * 64],
                            in_=ps_s.rearrange("p (m j) -> p j m", j=64),
                            axis=mybir.AxisListType.X, op=Alu.max)
                    m8w = tpool.tile([P, 8], f32, tag="m8w")
                    nc.vector.max(out=m8w[:], in_=ewin[:])
                    nc.vector.max_index(out=widx8p[g % 2][:, i - g0, :],
                                        in_max=m8w[:], in_values=ewin[:])

                # ============ Phase C: window coord gather ============
                idx16 = spool.tile([P, NW * G], i16, tag="idx16")
                nc.gpsimd.tensor_copy(
                    idx16.rearrange("p (j t) -> p j t", j=NW),
                    widx8p[g % 2].rearrange("p t j -> p j t")[:, 0:NW, :],
                )
                # q-major DRAM roundtrip: one write, one broadcast read
                nc.sync.dma_start(
                    d_scr_c[g].rearrange("q (r m) -> r q m", r=8), idx16[:])
                xc = spool.tile([P, 8, NW * G], i16, tag="xc")
                nc.sync.dma_start(
                    xc.rearrange("p r m -> p (r m)"),
                    d_scr_c[g].rearrange("q (o m) -> o q m", o=1)
                        .to_broadcast([8, 16, 8 * NW * G]))
                idx16c = spool.tile([P, NW, G, 8], i16, tag="idx16c")
                nc.gpsimd.tensor_copy(
                    idx16c.rearrange("p j t r -> p (j t) r"),
                    xc.rearrange("p r m -> p m r"),
                )
                cposJ = gpool.tile([P, NW, G, WSZ * 4], f32, tag="cposJ")
                for j in range(NW):
                    for c8 in range(0, G, 8):       # <=1024 idxs per gather
                        nc.gpsimd.dma_gather(
                            out_ap=cposJ[:, j, c8:c8 + 8],
                            in_ap=d_pswin,
                            idxs_ap=idx16c[:, j, c8:c8 + 8],
                            num_idxs=8 * P,
                            num_idxs_reg=8 * P,
                            elem_size=WSZ * 4,
                        )

                # ============ Phase D: exact refine over 64 candidates ======
                dxyz = gpool.tile([P, 3, G, NC], f32, tag="dxyz", bufs=2)
                cpv = cposJ.rearrange("p j t (m c) -> p t j m c", c=4)
                # fused (coord - pt)^2 on ACT via per-partition bias; exact
                # fp32 reference op order ((c + (-pt)) rounded, then squared)
                for i in range(g0, g0 + G):
                    for c in range(3):
                        nc.scalar.activation(
                            dxyz[:, c, i - g0].rearrange(
                                "p (j m) -> p j m", j=NW),
                            cpv[:, i - g0, :, :, c],
                            Act.Square, scale=1.0,
                            bias=nptq3[:, i, c:c + 1])
                ae = nc.vector if g >= n_grp - 2 else nc.gpsimd
                ae.tensor_tensor(
                    out=dxyz[:, 0], in0=dxyz[:, 0], in1=dxyz[:, 1], op=Alu.add)
                nd2 = gpool.tile([P, G, NC], f32, tag="nd2", bufs=2)
                ae.tensor_tensor(
                    out=nd2[:], in0=dxyz[:, 0], in1=dxyz[:, 2], op=Alu.add)
                ae.tensor_scalar(
                    out=nd2[:], in0=nd2[:], scalar1=-1.0, scalar2=None,
                    op0=Alu.mult)
                for i in range(g0, g0 + G):
                    nc.vector.max(out=snd2[:, i - g0, :], in_=nd2[:, i - g0])
                    nc.vector.max_index(out=slots[:, i - g0, :],
                                        in_max=snd2[:, i - g0, :],
                                        in_values=nd2[:, i - g0])

                # weights: w_k = 1/d2_k, normalized
                snd3 = snd2[:, :, 0:K]
                nc.vector.tensor_scalar(w3[:], snd3, -1.0, scalar2=None,
                                        op0=Alu.mult)
                nc.vector.reciprocal(w3[:], w3[:])
                nc.vector.tensor_tensor(sumw[:], w3[:, :, 0], w3[:, :, 1],
                                        op=Alu.add)
                nc.vector.tensor_tensor(sumw[:], sumw[:], w3[:, :, 2],
                                        op=Alu.add)
                nc.vector.reciprocal(rsum[:], sumw[:])
                for k in range(K):
                    nc.vector.tensor_tensor(wn[:, :, k], w3[:, :, k],
                                            rsum[:], op=Alu.mult)

                # slot -> source index: src = widx[w] + 128*m,  w=slot//16,
                # m=slot%16.  All in f32 mask arithmetic.
                nc.gpsimd.tensor_copy(slotf[:], slots[:, :, 0:K])
                nc.gpsimd.tensor_copy(widxf[:], widx8[:, :, 0:NW])
                # t = (slot - 7.5)/16 lands within +-0.47 of the window id j
                de = nc.gpsimd
                de.tensor_scalar(tsel[:], slotf[:], -7.5, 0.0625,
                                 op0=Alu.add, op1=Alu.mult)
                # source = base(widx) + 64*m with base = widx%64 +
                # 1024*[widx>=64] = widx + 960*[widx>=64];  since
                # slot = 16*w + m:  src = 64*slot + 1[w==j]*(base_j - 1024*j)
                de.tensor_scalar(basel[:], widxf[:], 63.5, 960.0,
                                 op0=Alu.is_ge, op1=Alu.mult)
                de.tensor_tensor(basel[:], widxf[:], basel[:], op=Alu.add)
                de.tensor_scalar(srcf[:], slotf[:], 64.0, scalar2=None,
                                 op0=Alu.mult)
                for j in range(NW):
                    de.tensor_scalar(widxadj[:, :, j], basel[:, :, j],
                                     -1024.0 * j, scalar2=None,
                                     op0=Alu.add)
                for j in range(NW):
                    de.tensor_scalar(c1m[:], tsel[:], j - 0.5,
                                     scalar2=None, op0=Alu.is_ge)
                    de.tensor_scalar(c2m[:], tsel[:], j + 0.5,
                                     scalar2=None, op0=Alu.is_lt)
                    de.tensor_tensor(c1m[:], c1m[:], c2m[:], op=Alu.mult)
                    de.tensor_tensor(
                        accw[:], c1m[:],
                        widxadj[:, :, j:j + 1].to_broadcast([P, G, K]),
                        op=Alu.mult)
                    de.tensor_tensor(srcf[:], srcf[:], accw[:], op=Alu.add)
                nc.gpsimd.tensor_copy(
                    srci[g % 2].rearrange("p (t k) -> p t k", k=K), srcf[:])
                # diag(wn) blocks for the weight matmuls (off the EF chain)
                diag3 = gpool.tile([P, G, K, P], bf16, tag="diag3", bufs=2)
                diag_bufs[g] = diag3
                if g >= n_grp - 2:
                    # tail groups: DVE is idle there, ACT is the pacer
                    for i in range(g0, g0 + G):
                        nc.vector.tensor_tensor(
                            out=diag3[:, i - g0],
                            in0=id3b.rearrange("p (k f) -> p k f", k=K),
                            in1=wn[:, i - g0:i - g0 + 1, :]
                                .rearrange("p o k -> p k o")
                                .to_broadcast([P, K, P]),
                            op=Alu.mult)
                else:
                    for i in range(g0, g0 + G):
                        for k in range(K):
                            nc.scalar.activation(
                                diag3[:, i - g0, k], ident[:],
                                Act.Copy, scale=wn[:, i - g0, k:k + 1])

                # ============ Phase E/F: feature gather + interp + MLP ======
                sidx16 = spool.tile([P, G * K], i16, tag="sidx16")
                nc.gpsimd.tensor_copy(sidx16[:], srci[g % 2][:])
                nc.sync.dma_start(
                    d_scr_f[g].rearrange("q (r m) -> r q m", r=8), sidx16[:])
                xf = spool.tile([P, 8, G * K], i16, tag="xf")
                nc.sync.dma_start(
                    xf.rearrange("p r m -> p (r m)"),
                    d_scr_f[g].rearrange("q (o m) -> o q m", o=1)
                        .to_broadcast([8, 16, 8 * G * K]))
                idx16f = spool.tile([P, G * K, 8], i16, tag="idx16f")
                nc.gpsimd.tensor_copy(idx16f[:], xf.rearrange("p r m -> p m r"))
                for fg0 in range(0, G, fg):
                    # one tile per 1024-idx gather so early MLP pairs only
                    # depend on the first gather, not the whole batch
                    gfs = [gpool.tile([P, 8, C_SRC], bf16, tag=f"gf{c}",
                                      bufs=2, name=f"gf{c}")
                           for c in range(3)]
                    gf_bufs[(g, fg0)] = gfs
                    for c, lo in enumerate(range(0, fg * K * P, 1024)):
                        nc.gpsimd.dma_gather(
                            out_ap=gfs[c][:],
                            in_ap=d_xs,
                            idxs_ap=idx16f[:, fg0 * K + lo // P:
                                           fg0 * K + lo // P + 8],
                            num_idxs=1024,
                            num_idxs_reg=1024,
                            elem_size=C_SRC,
                        )

            def emit_ef_mlp(g, chunk):
                _mark(f"EFm{g}")
                g0 = g * G
                for fg0 in (chunk,):
                    gfs = gf_bufs.pop((g, fg0))
                    ct0s = {}
                    for pp in range(0, fg, 2):      # xtT loads have no deps
                        ct0 = mpool.tile([P, 2 * P], f32r, tag="ct0", bufs=4)
                        i0 = g0 + fg0 + pp
                        nc.sync.dma_start(ct0[:], d_xtT[:, i0 * P:(i0 + 2) * P])
                        ct0s[pp] = ct0
                    for pp in range(0, fg, 2):      # tile pairs -> N=256
                        i0 = g0 + fg0 + pp
                        it2 = psm.tile([P, 2, 2 * P], f32, tag="it2", bufs=2)
                        it_lo = it2[:, 0]
                        it_hi = it2[:, 1]
                        diag3 = diag_bufs[g]
                        _lp = tc.high_priority(offset=-50000)
                        _lp.__enter__()
                        for half, it_h in ((0, it_lo), (1, it_hi)):
                            for u in range(2):
                                ii = pp + u
                                tl = fg0 + pp + u
                                for k in range(K):
                                    r = ii * K + k
                                    nc.tensor.matmul(
                                        it_h[:, u * P:(u + 1) * P],
                                        lhsT=gfs[r // 8][:, r % 8,
                                                half * P:(half + 1) * P],
                                        rhs=diag3[:, tl, k, :],
                                        start=(k == 0), stop=(k == K - 1),
                                    )
                        _lp.__exit__(None, None, None)
                        ct0 = ct0s[pp]
                        ct1 = mpool.tile([P, 2 * P], f32r, tag="ct1")
                        ct2 = mpool.tile([P, 2 * P], f32r, tag="ct2")
                        if g >= n_grp - 2:
                            nc.vector.tensor_copy(ct1[:], it_lo[:])
                            nc.vector.tensor_copy(ct2[:], it_hi[:])
                        else:
                            nc.scalar.activation(ct1[:], it_lo[:], Act.Copy)
                            nc.scalar.activation(ct2[:], it_hi[:], Act.Copy)
                        cts = (ct0, ct1, ct2)
                        ps_h = psm.tile([P, 2, 2 * P], f32, tag="ph", bufs=1)
                        for m in range(2):
                            for k in range(3):
                                nc.tensor.matmul(
                                    ps_h[:, m, :],
                                    lhsT=w1[:, (k * 2 + m) * P:(k * 2 + m + 1) * P],
                                    rhs=cts[k][:],
                                    start=(k == 0), stop=(k == 2),
                                )
                        hs = mpool.tile([P, 2, 2 * P], f32r, tag="hs")
                        for m in range(2):
                            if g >= n_grp - 2:
                                nc.vector.tensor_scalar(
                                    hs[:, m, :], ps_h[:, m, :],
                                    b1[:, m:m + 1], 0.0,
                                    op0=Alu.add, op1=Alu.max)
                            else:
                                nc.scalar.activation(
                                    hs[:, m, :], ps_h[:, m, :],
                                    Act.Relu, bias=b1[:, m:m + 1],
                                )
                        ps_o = psm.tile([P, 2 * P], f32, tag="po", bufs=1)
                        for k in range(2):
                            nc.tensor.matmul(
                                ps_o[:], lhsT=w2[:, k * P:(k + 1) * P],
                                rhs=hs[:, k, :], start=(k == 0), stop=False,
                            )
                        for k in range(3):
                            nc.tensor.matmul(
                                ps_o[:], lhsT=ws[:, k * P:(k + 1) * P],
                                rhs=cts[k][:], start=False, stop=(k == 2),
                            )
                        ot = mpool.tile([P, 2 * P], f32, tag="ot")
                        nc.scalar.activation(ot[:], ps_o[:], Act.Relu,
                                             bias=bo[:, 0:1])
                        nc.sync.dma_start(d_out[:, i0 * P:(i0 + 2) * P], ot[:])

    nc.compile()
    return nc


def host_prep(inputs, tt=TT):
    """Build the per-core input maps (layout-only host prep)."""
    nt = tt * P
    x_target = np.asarray(inputs["x_target"], np.float32)
    pos_target = np.asarray(inputs["pos_target"], np.float32)
    x_source = np.asarray(inputs["x_source"], np.float32)
    pos_source = np.asarray(inputs["pos_source"], np.float32)
    W1 = np.asarray(inputs["W1"], np.float32)
    b1 = np.asarray(inputs["b1"], np.float32)
    W2 = np.asarray(inputs["W2"], np.float32)
    b2 = np.asarray(inputs["b2"], np.float32)
    Ws = np.asarray(inputs["Ws"], np.float32)
    bs = np.asarray(inputs["bs"], np.float32)

    w1t = W1.reshape(3, P, 2, P).transpose(1, 0, 2, 3).reshape(P, 3 * 2 * P).copy()
    w2t = W2.reshape(2, P, P).transpose(1, 0, 2).reshape(P, 2 * P).copy()
    wst = Ws.reshape(3, P, P).transpose(1, 0, 2).reshape(P, 3 * P).copy()
    b1t = b1.reshape(2, P).T.copy()
    bot = (b2 + bs).reshape(P, 1).copy()
    ident = np.eye(P, dtype=np.float32)
    id3b = np.tile(np.eye(P, dtype=np.float32), (1, 3)).reshape(P, 3 * P)
    id3b = np.asarray(id3b, ml_dtypes.bfloat16)

    in_maps = []
    for c in range(B):
        pt = pos_target[c * NT:c * NT + nt]
        ps = pos_source[c * NS:(c + 1) * NS]
        a_hi, a_lo = _bf16_split(pt)
        b_hi, b_lo = _bf16_split(ps)
        q = -0.5 * (ps.astype(np.float64) ** 2).sum(-1)
        q = q.astype(np.float32)
        q_hi, q_lo = _bf16_split(q)
        one = np.ones(nt, ml_dtypes.bfloat16)
        zero = np.zeros(nt, ml_dtypes.bfloat16)
        lhsT = np.stack(
            [a_hi[:, 0], a_hi[:, 0], a_lo[:, 0],
             a_hi[:, 1], a_hi[:, 1], a_lo[:, 1],
             a_hi[:, 2], a_hi[:, 2], a_lo[:, 2],
             one, one, zero], axis=0)
        zs = np.zeros(NS, ml_dtypes.bfloat16)
        rhs = np.stack(
            [b_hi[:, 0], b_lo[:, 0], b_hi[:, 0],
             b_hi[:, 1], b_lo[:, 1], b_hi[:, 1],
             b_hi[:, 2], b_lo[:, 2], b_hi[:, 2],
             q_hi, q_lo, zs], axis=0)
        ptT = pt.reshape(tt, P, 3).transpose(1, 0, 2).reshape(P, tt * 3).copy()
        nptq = (-0.5 * (pt.astype(np.float32) ** 2).sum(
            -1, dtype=np.float32)).reshape(tt, P).T.copy()
        nptc = (-pt).reshape(tt, P, 3).transpose(1, 0, 2).reshape(P, tt * 3).copy()
        xtT = x_target[c * NT:c * NT + nt].T.copy()
        # window table: ps_win[j, m] = ps[j + 128m] padded to 16B rows
        ps_win = np.zeros((NWIN, WSZ, 4), np.float32)
        ps_win[:, :, :3] = ps.reshape(2, WSZ, 64, 3).transpose(
            0, 2, 1, 3).reshape(NWIN, WSZ, 3)
        xs = np.asarray(x_source[c * NS:(c + 1) * NS], ml_dtypes.bfloat16)
        in_maps.append({
            "lhsT_pt": lhsT, "rhs_ps": rhs, "ptT": ptT, "xtT": xtT,
            "nptq": nptq, "nptc": nptc,
            "ps_win": ps_win.reshape(NWIN, WSZ * 4), "xs": xs,
            "w1t": w1t, "w2t": w2t, "wst": wst, "b1t": b1t, "bot": bot,
            "ident": ident, "id3b": id3b,
        })
    return in_maps


_CACHED = {}
LAST_RESULT = None


def kernel(**inputs):
    global LAST_RESULT
    from concourse import bass_utils

    if "nc" not in _CACHED:
        _CACHED["nc"] = build_program(TT)
    nc = _CACHED["nc"]
    in_maps = host_prep(inputs, TT)
    res = bass_utils.run_bass_kernel_spmd(nc, in_maps, core_ids=list(range(B)))
    LAST_RESULT = res
    outs = []
    for c in range(B):
        outT = res.results[c]["outT"]
        outs.append(np.ascontiguousarray(outT.T))
    return np.concatenate(outs, axis=0)
